# revision 38
# baseline (speedup 1.0000x reference)
"""Self-contained Trainium2 Bass kernel for masked multi-head attention.

Problem: B=8, L=1024, D=1024, H=16, DK=DV=64, fp32, pad-masked softmax.
Returns (out [B,L,D], attn [B,H,L,L]) matching the jax reference.

Strategy: data-parallel over batch B across the 8 NeuronCores. Each core runs
the full attention for one batch element. All matmuls run as float32r (full
PE rate at N>=256). The pad mask is folded in three ways:
  - key mask on S:  augmented contraction row (ones x -240000*padk), K=65
  - query mask on P = exp(S): per-partition bias AP on the ScalarE exp
  - key mask on P^T = exp(S^T): per-partition bias AP on the second exp
Row sums come free via the ScalarE activation accum_out. The second matmul
O^T = V^T P^T uses V as the stationary operand (natural layout from the
V projection) so no transposes of the 16M-element attention matrix are ever
needed; instead S^T is computed directly by swapping the QK matmul operands.
Host-side prep only re-lays-out inputs (transposes of X/W weights).

Per-core schedule (Tile cost model): ~423 us against per-engine busy
floors of ~290 us (ScalarE exps), ~265 us (DMA: 20 MB in + 68 MB out),
~260 us (PE fp32r matmuls). The pair loop software-pipelines weight
loads and Q/K projections one pair ahead (DMA prefetches issued during
the store-free C phase so the attn-store stream keeps HBM saturated
during B), and the O^T accumulation runs one step behind the S^T
production so no in-order PE stream ever parks behind an exp wait.
Accuracy vs a float64 reference: attn ~2.6e-4 rel, out ~2.3e-3 rel
(out passes through the bf16 O^T path; attn is fp32r/fp32 end-to-end).
"""

from contextlib import ExitStack

import os

import numpy as np

import concourse.bass as bass
import concourse.tile as tile
from concourse import mybir
import bass_rust
from concourse.bass_utils import run_bass_kernel_spmd

F32 = mybir.dt.float32
F32R = mybir.dt.float32r
BF16 = mybir.dt.bfloat16
EXP = mybir.ActivationFunctionType.Exp

B, L, D, H, DK, DV = 8, 1024, 1024, 16, 64, 64
NP = 8          # head pairs
NQ = 4          # head quads
NT = 8          # 128-row tiles per L
SCALE = 0.125   # 1/sqrt(DK)
BIGK = -240000.0   # key-mask addend inside S (pre-scale)
BIGQ = -30000.0    # query-mask bias on exp (post-scale)

_wsplit_ctr = [0]


def _split_multi_waits(nc, max_waits=1):
    """This container's walrus only accepts one sync-wait command per
    instruction; hoist extra waits onto NOPs inserted just before."""
    for f in nc.m.functions:
        for bb in f.blocks:
            insts = list(bb.instructions)
            new, changed = [], False
            for inst in insts:
                si = inst.sync_info
                if si is not None:
                    waits = list(si.on_wait)
                    if len(waits) > max_waits:
                        for w in waits[:-max_waits]:
                            _wsplit_ctr[0] += 1
                            nop = mybir.InstNoOp(
                                name=f"I-wsplit-{_wsplit_ctr[0]}", ins=[], outs=[])
                            nop.engine = inst.engine
                            nop.sync_info = bass_rust.SyncInfo(
                                on_wait=[w], on_update=[])
                            new.append(nop)
                        inst.sync_info = bass_rust.SyncInfo(
                            on_wait=waits[-max_waits:],
                            on_update=list(si.on_update))
                        changed = True
                new.append(inst)
            if changed:
                bb.instructions = new


def _r(ap):
    return ap


def build_nc():
    nc = bass.Bass("TRN2", target_bir_lowering=False, debug=False,
                   num_devices=1)

    xt_d = nc.dram_tensor("xt", [D, L], F32R, kind="ExternalInput")
    wqt_d = nc.dram_tensor("wqt", [NP, D, 128], F32R, kind="ExternalInput")
    wkt_d = nc.dram_tensor("wkt", [NP, D, 128], F32R, kind="ExternalInput")
    wvt_d = nc.dram_tensor("wvt", [NQ, D, 256], F32R, kind="ExternalInput")
    wot_d = nc.dram_tensor("wot", [D, D], F32R, kind="ExternalInput")
    qbias_d = nc.dram_tensor("qbias", [128, NT], F32, kind="ExternalInput")
    rmask_d = nc.dram_tensor("rmask", [128, NT], F32, kind="ExternalInput")
    ones_row_d = nc.dram_tensor("ones_row", [1, L], F32R, kind="ExternalInput")
    kmask_row_d = nc.dram_tensor("kmask_row", [1, L], F32R, kind="ExternalInput")
    ones_col_d = nc.dram_tensor("ones_col", [1, 64], F32R, kind="ExternalInput")

    out_d = nc.dram_tensor("out", [L, D], F32, kind="ExternalOutput")
    attn_d = nc.dram_tensor("attn", [H, L, L], F32, kind="ExternalOutput")

    with tile.TileContext(nc) as tc:
        with ExitStack() as stack:
            ent = stack.enter_context
            # ---------- persistent pools (live whole kernel) ----------
            catt_pool = ent(tc.tile_pool(name="catt", bufs=NP))
            small_pool = ent(tc.tile_pool(name="small", bufs=1))
            rsum_pool = ent(tc.tile_pool(name="rsum", bufs=2))
            rflat_pool = ent(tc.tile_pool(name="rflat", bufs=1))
            qbias_t = small_pool.tile([128, NT], F32)
            nc.sync.dma_start(qbias_t[:], qbias_d[:])
            rmask_t = small_pool.tile([128, NT], F32)
            nc.sync.dma_start(rmask_t[:], rmask_d[:])
            ones_row_t = small_pool.tile([1, L], F32R)
            nc.sync.dma_start(ones_row_t[:], ones_row_d[:])
            kmask_row_t = small_pool.tile([1, L], F32R)
            nc.sync.dma_start(kmask_row_t[:], kmask_row_d[:])
            ones_col_t = small_pool.tile([1, 64], F32R)
            nc.sync.dma_start(ones_col_t[:], ones_col_d[:])

            warm = small_pool.tile([128, NT], F32, name="warm")
            nc.scalar.activation(warm[:], qbias_t[:], EXP, bias=0.0,
                                 scale=0.0)


            catt = []  # O^T (normalized) per pair: [128 (hv), L(q)]
            for p in range(NP):
                catt.append(catt_pool.tile([128, L], F32R, name=f"catt{p}", tag=f"catt{p}", bufs=1))

            with ExitStack() as pair_stack:
                pent = pair_stack.enter_context
                # ---------- pair-phase pools ----------
                xt_pool = pent(tc.tile_pool(name="xt", bufs=1))
                wq_pool = pent(tc.tile_pool(name="wq", bufs=2))
                wk_pool = pent(tc.tile_pool(name="wk", bufs=2))
                wvt_pool = pent(tc.tile_pool(name="wvt", bufs=2))
                vsb_pool = pent(tc.tile_pool(name="vsb", bufs=2))
                qaug_pool = pent(tc.tile_pool(name="qaug", bufs=2))
                kaug_pool = pent(tc.tile_pool(name="kaug", bufs=2))
                p_pool = pent(tc.tile_pool(name="pp", bufs=8))
                pt_pool = pent(tc.tile_pool(name="pt", bufs=3))
                rr_pool = pent(tc.tile_pool(name="rr", bufs=1))
                ps_m = pent(tc.tile_pool(name="ps_m", bufs=3, space="PSUM"))
                ps_o = pent(tc.tile_pool(name="ps_o", bufs=1, space="PSUM"))
                xt_big = xt_pool.tile([128, NT * L], F32R, name="xt_big",
                                      tag="xt")
                nc.sync.dma_start(
                    xt_big[:],
                    xt_d.rearrange("(c p) l -> p c l", p=128))
                xt = [xt_big[:, c * L:(c + 1) * L] for c in range(NT)]

                def v_parts(g):
                    """V projection for quad g, split into a load closure
                    (issued during the preceding C phase) and per-st matmul
                    fillers (run during the odd pair's B phase)."""
                    res = {"vsb": []}

                    def loads():
                        wvt_big = wvt_pool.tile(
                            [128, NT * 256], F32R, name=f"wvt{g}", tag="wvt")
                        nc.sync.dma_start(
                            wvt_big[:],
                            wvt_d[g].rearrange("(c p) j -> p c j", p=128))
                        res["wvt"] = wvt_big

                    def mk(st):
                        def run():
                            wvt_big = res["wvt"]
                            v_ps = ps_m.tile([128, L], F32,
                                             name=f"vps{g}_{st}",
                                             tag="m")[:, 0:256]
                            for c in range(NT):
                                nc.tensor.matmul(
                                    v_ps[:],
                                    _r(xt[c][:, st * 128:(st + 1) * 128]),
                                    _r(wvt_big[:, c * 256:(c + 1) * 256]),
                                    start=(c == 0), stop=(c == NT - 1))
                            v_t = vsb_pool.tile([128, 256], BF16,
                                                name=f"vsb{g}_{st}",
                                                tag=f"vsb{st}")
                            nc.vector.tensor_copy(v_t[:], v_ps[:])
                            res["vsb"].append(v_t)
                        return run
                    return loads, [mk(st) for st in range(NT)], res

                def qk_parts(p):
                    res = {}

                    def loads():
                        res["wq"] = _project_load(nc, p, wq_pool, wqt_d, "q")
                        res["wk"] = _project_load(nc, p, wk_pool, wkt_d, "k")

                    def mm_q():
                        res["q"] = _project(nc, p, xt, qaug_pool, res["wq"],
                                            ones_row_t[:, :], ps_m, "q")

                    def mm_k():
                        res["k"] = _project(nc, p, xt, kaug_pool, res["wk"],
                                            kmask_row_t[:, :], ps_m, "k")
                    return loads, [mm_q, mm_k], res

                # spin the PE while input DMAs stream so the HAM clock
                # gate reaches 2.4 GHz before the first projection
                warm_ps = ps_m.tile([64, L], F32, name="warm_ps", tag="m")
                for w in range(24):
                    nc.tensor.matmul(warm_ps[:, 0:512],
                                     _r(ones_col_t[:, :]),
                                     _r(ones_row_t[:, 0:512]),
                                     start=True, stop=True)

                # prologue: pair 0 + quad 0 eagerly, pair 1 loads eagerly
                l0, m0, r0 = qk_parts(0)
                l0()
                for m in m0:
                    m()
                qk = (r0["q"], r0["k"])
                vl0, vm0, vr0 = v_parts(0)
                vl0()
                for m in vm0:
                    m()
                vsb_cur = vr0["vsb"]

                nxt_qk = None
                if NP > 1:
                    l1, m1, r1 = qk_parts(1)
                    l1()
                    nxt_qk = (m1, r1)
                nxt_v = None

                deferred = []
                for p in range(NP):
                    g = p // 2
                    fillers = list(deferred)
                    preloads = []
                    if nxt_qk is not None:
                        fillers += nxt_qk[0]
                    if p % 2 == 1 and nxt_v is not None:
                        fillers += nxt_v[0]
                    # loads for work whose matmul fillers run next pair
                    nxt2_qk = None
                    if p + 2 < NP:
                        l2, m2, r2 = qk_parts(p + 2)
                        preloads.append(l2)
                        nxt2_qk = (m2, r2)
                    if p % 2 == 0 and g + 1 < NQ:
                        vl, vm, vr = v_parts(g + 1)
                        preloads.append(vl)
                        nxt_v = (vm, vr)
                    deferred = _attend_pair(
                        nc, p, qk, vsb_cur,
                        p_pool, pt_pool, rr_pool,
                        rsum_pool, rflat_pool, ps_m, ps_o,
                        attn_d, qbias_t, rmask_t, ones_col_t,
                        catt[p], fillers, preloads)
                    while fillers:
                        fillers.pop(0)()
                    if nxt_qk is not None:
                        qk = (nxt_qk[1]["q"], nxt_qk[1]["k"])
                    nxt_qk = nxt2_qk
                    if p % 2 == 1 and nxt_v is not None:
                        vsb_cur = nxt_v[1]["vsb"]
                for f in deferred:
                    f()

            # ---------- final output projection ----------
            with ExitStack() as fin_stack:
                fent = fin_stack.enter_context
                wot_pool = fent(tc.tile_pool(name="wot", bufs=1))
                outsb_pool = fent(tc.tile_pool(name="outsb", bufs=2))
                ps_f = fent(tc.tile_pool(name="ps_f", bufs=2, space="PSUM"))
                wot_big = wot_pool.tile([128, NP * D], F32R, name="wot_big",
                                        tag="wot")
                wot_r = wot_d.rearrange("(c p) l -> p c l", p=128)
                nc.sync.dma_start(wot_big[:, 0:4 * D], wot_r[:, 0:4, :])
                nc.sync.dma_start(wot_big[:, 4 * D:8 * D], wot_r[:, 4:8, :])
                wot = [wot_big[:, p * D:(p + 1) * D] for p in range(NP)]
                for qt in range(NT):
                    o_ps = ps_f.tile([128, D], F32)
                    for dh in range(2):
                        for p in range(NP):
                            nc.tensor.matmul(
                                o_ps[:, dh * 512:(dh + 1) * 512],
                                _r(catt[p][:, qt * 128:(qt + 1) * 128]),
                                _r(wot[p][:, dh * 512:(dh + 1) * 512]),
                                start=(p == 0), stop=(p == NP - 1))
                    o_sb = outsb_pool.tile([128, D], F32)
                    nc.vector.tensor_copy(o_sb[:], o_ps[:])
                    nc.gpsimd.dma_start(
                        out_d[qt * 128:(qt + 1) * 128, :], o_sb[:])

    _split_multi_waits(nc)
    return nc


def _project_load(nc, p, w_pool, w_dram, kind):
    w_big = w_pool.tile([128, NT * 128], F32R, name=f"w{kind}{p}",
                        tag=f"w{kind}")
    nc.sync.dma_start(
        w_big[:], w_dram[p].rearrange("(c q) j -> q c j", q=128))
    return w_big


def _project(nc, p, xt, aug_pool, w_big, mask_row, ps_m, kind):
    """Q^T/K^T pair projection -> two augmented per-head tiles [65, L]
    (rows 0-63: head data, row 64: ones / -BIGK*padk)."""
    wts = [w_big[:, c * 128:(c + 1) * 128] for c in range(NT)]
    pr_ps = ps_m.tile([128, L], F32, name=f"prps{kind}{p}", tag="m")
    for lh in range(2):
        for c in range(NT):
            nc.tensor.matmul(
                pr_ps[:, lh * 512:(lh + 1) * 512],
                _r(wts[c][:]),
                _r(xt[c][:, lh * 512:(lh + 1) * 512]),
                start=(c == 0), stop=(c == NT - 1))
    augs = []
    for hi in range(2):
        aug = aug_pool.tile([65, L], F32R, name=f"aug{kind}{p}_{hi}",
                            tag=f"aug{hi}")
        nc.vector.tensor_copy(aug[0:64, :],
                              pr_ps[hi * 64:(hi + 1) * 64, :])
        nc.sync.dma_start(aug[64:65, :], mask_row)
        augs.append(aug)
    return augs


def _attend_pair(nc, p, qk, vsb,
                 p_pool, pt_pool, rr_pool, rsum_pool, rflat_pool,
                 ps_m, ps_o, attn_d, qbias_t, rmask_t, ones_col_t, catt_p,
                 fillers=(), preloads=()):
    fillers = list(fillers) if not isinstance(fillers, list) else fillers
    qaug, kaug = qk
    h0, h1 = 2 * p, 2 * p + 1

    rsum = [rsum_pool.tile([128, NT], F32, name=f"rsum{p}_{i}", tag=f"rsum{i}")
            for i in range(2)]
    rstar = [rsum_pool.tile([128, NT], F32, name=f"rstar{p}_{i}",
                            tag=f"rstar{i}")
             for i in range(2)]
    rstar_r = [rsum_pool.tile([128, NT], F32R, name=f"rstarr{p}_{i}",
                              tag=f"rstarr{i}")
               for i in range(2)]

    # ---- S -> exp -> A -> store (B phase) ----
    for hi in range(2):
        h = h0 if hi == 0 else h1
        for qt in range(NT):
            s_ps = ps_m.tile([128, L], F32, tag="m")
            for sh in range(2):
                nc.tensor.matmul(
                    s_ps[:, sh * 512:(sh + 1) * 512],
                    _r(qaug[hi][:, qt * 128:(qt + 1) * 128]),
                    _r(kaug[hi][:, sh * 512:(sh + 1) * 512]),
                    start=True, stop=True)
            p_t = p_pool.tile([128, L], F32)
            nc.scalar.activation(p_t[:], s_ps[:], EXP,
                                 bias=qbias_t[:, qt:qt + 1], scale=SCALE,
                                 accum_out=rsum[hi][:, qt:qt + 1])
            nc.vector.tensor_scalar_add(rstar[hi][:, qt:qt + 1],
                                        rsum[hi][:, qt:qt + 1], 1e-12)
            nc.vector.reciprocal(rstar[hi][:, qt:qt + 1],
                                 rstar[hi][:, qt:qt + 1])
            nc.vector.tensor_mul(rstar[hi][:, qt:qt + 1],
                                 rstar[hi][:, qt:qt + 1],
                                 rmask_t[:, qt:qt + 1])
            nc.vector.tensor_scalar_mul(p_t[:], p_t[:],
                                        rstar[hi][:, qt:qt + 1])
            nc.sync.dma_start(attn_d[h, qt * 128:(qt + 1) * 128, :],
                              p_t[:])
            if fillers and (hi, qt) >= (0, 2):
                fillers.pop(0)()

    # C phase start: issue prefetch DMAs for the next pair/quad plus the
    # r-flatten DMAs of this pair (the B phase saturates HBM with stores).
    rflat = [rflat_pool.tile([1, L], F32R, name=f"rflat{p}_{hi2}",
                             tag=f"rflat{hi2}")
             for hi2 in range(2)]

    def d_loads():
        for hi2 in range(2):
            nc.vector.tensor_copy(rstar_r[hi2][:], rstar[hi2][:])
            for t in range(NT):
                nc.sync.dma_start(rflat[hi2][:, t * 128:(t + 1) * 128],
                                  rstar_r[hi2][:, t:t + 1])
    for f in preloads:
        f()
    d_loads()

    # ---- S^T -> exp -> P^T ; O^T accumulation (C phase; bf16 O^T packs
    # both heads into one tile via column tile_position). The O^T matmuls
    # for step st are emitted AFTER the S^T matmuls of st+1: engines issue
    # in order, so an O^T waiting on exp2(st) must not sit ahead of ready
    # S^T work in the PE stream. ----
    ot_ps = ps_o.tile([128, L], F32, name=f"ot{p}", tag="ot")
    pt_hist = {}

    def st_mms(st):
        st_ps = [ps_m.tile([128, L], F32, name=f"stps{p}_{st}_{i}", tag="m")
                 for i in range(2)]
        for hi in range(2):
            for qh in range(2):
                nc.tensor.matmul(
                    st_ps[hi][:, qh * 512:(qh + 1) * 512],
                    _r(kaug[hi][:, st * 128:(st + 1) * 128]),
                    _r(qaug[hi][:, qh * 512:(qh + 1) * 512]),
                    start=True, stop=True)
        pt_t = [pt_pool.tile([128, L], BF16, name=f"pt{p}_{st}_{i}",
                             tag=f"pt{i}")
                for i in range(2)]
        for hi in range(2):
            nc.scalar.activation(pt_t[hi][:], st_ps[hi][:], EXP,
                                 bias=0.0, scale=SCALE)
        pt_hist[st] = pt_t

    def ot_mms(st):
        pt_t = pt_hist.pop(st)
        for qh in range(2):
            for hi in range(2):
                nc.tensor.matmul(
                    ot_ps[hi * 64:(hi + 1) * 64, qh * 512:(qh + 1) * 512],
                    vsb[st][:, ((p % 2) * 2 + hi) * 64:
                            ((p % 2) * 2 + hi + 1) * 64],
                    pt_t[hi][:, qh * 512:(qh + 1) * 512],
                    start=(st == 0), stop=(st == NT - 1),
                    tile_position=(0, hi * 64))

    for st in range(NT):
        st_mms(st)
        if st >= 1:
            ot_mms(st - 1)

    # ---- normalize O^T by r (replicated along partitions). Deferred:
    # returned as closures the caller interleaves into the NEXT pair's B
    # phase, keeping this off the inter-pair critical path. ----
    def d_phase(hi):
        def run():
            rr_ps = ps_m.tile([64, L], F32, name=f"rrps{p}_{hi}", tag="m")
            for qh in range(2):
                nc.tensor.matmul(
                    rr_ps[:, qh * 512:(qh + 1) * 512],
                    _r(ones_col_t[:, :]),
                    _r(rflat[hi][:, qh * 512:(qh + 1) * 512]),
                    start=True, stop=True)
            rr_sb = rr_pool.tile([64, L], F32, name=f"rrsb{p}_{hi}",
                                 tag=f"rrsb{hi}")
            nc.vector.tensor_copy(rr_sb[:], rr_ps[:])
            nc.vector.tensor_mul(catt_p[hi * 64:(hi + 1) * 64, :],
                                 ot_ps[hi * 64:(hi + 1) * 64, :], rr_sb[:])
        return run

    def ot_tail():
        ot_mms(NT - 1)
    return [ot_tail, d_phase(0), d_phase(1)]


_CACHED = {}
_LAST_EXEC_NS = None
_LAST_RES = None


def _get_nc():
    if "nc" not in _CACHED:
        _CACHED["nc"] = build_nc()
    return _CACHED["nc"]


def _prep_core_inputs(Xi, WQT, WKT, WVT, WOT, pmi,
                      ones_row, ones_col):
    pm = pmi.astype(np.float32)
    pm_tiled = pm.reshape(NT, 128).T.copy()      # [128, NT]
    return {
        "xt": np.ascontiguousarray(Xi.T),
        "wqt": WQT, "wkt": WKT, "wvt": WVT, "wot": WOT,
        "qbias": np.ascontiguousarray(BIGQ * pm_tiled),
        "rmask": np.ascontiguousarray(1.0 - pm_tiled),
        "ones_row": ones_row,
        "kmask_row": np.ascontiguousarray((BIGK * pm)[None, :]),
        "ones_col": ones_col,
    }


def kernel(X, WQ, WK, WV, WO, pad_mask):
    X = np.asarray(X, dtype=np.float32)
    WQ = np.asarray(WQ, dtype=np.float32)
    WK = np.asarray(WK, dtype=np.float32)
    WV = np.asarray(WV, dtype=np.float32)
    WO = np.asarray(WO, dtype=np.float32)
    pad_mask = np.asarray(pad_mask)

    WQT = np.stack([np.concatenate([WQ[2 * p].T, WQ[2 * p + 1].T], axis=1)
                    for p in range(NP)])          # [NP, D, 128]
    WKT = np.stack([np.concatenate([WK[2 * p].T, WK[2 * p + 1].T], axis=1)
                    for p in range(NP)])
    WVT = np.stack([np.concatenate([WV[4 * g + j].T for j in range(4)], axis=1)
                    for g in range(NQ)])          # [NQ, D, 256]
    WOT = np.ascontiguousarray(WO.T)
    ones_row = np.ones((1, L), np.float32)
    ones_col = np.ones((1, 64), np.float32)

    in_maps = [
        _prep_core_inputs(X[i], WQT, WKT, WVT, WOT, pad_mask[i],
                          ones_row, ones_col)
        for i in range(B)
    ]

    nc = _get_nc()
    trace = bool(os.environ.get("KERNEL_TRACE"))
    kw = {}
    if trace:
        kw["trace"] = True
        kw["tmpdir"] = os.environ.get("KERNEL_TRACE_DIR") or None
    res = run_bass_kernel_spmd(nc, in_maps, list(range(B)), **kw)
    global _LAST_EXEC_NS, _LAST_RES
    _LAST_RES = res
    _LAST_EXEC_NS = res.exec_time_ns

    out = np.stack([res.results[i]["out"] for i in range(B)])
    attn = np.stack([res.results[i]["attn"] for i in range(B)])
    return out, attn


if __name__ == "__main__":
    rng = np.random.default_rng(0)
    X = rng.standard_normal((B, L, D), dtype=np.float32)
    WQ = (rng.standard_normal((H, DK, D), dtype=np.float32) * 0.02)
    WK = (rng.standard_normal((H, DK, D), dtype=np.float32) * 0.02)
    WV = (rng.standard_normal((H, DV, D), dtype=np.float32) * 0.02)
    WO = (rng.standard_normal((D, H * DV), dtype=np.float32) * 0.02)
    pm = rng.integers(0, 2, size=(B, L)).astype(bool)
    out, attn = kernel(X=X, WQ=WQ, WK=WK, WV=WV, WO=WO, pad_mask=pm)
    print("out", out.shape, "attn", attn.shape)


def bench(reps=16, **inputs):
    """Time repeated on-device executions with resident inputs.

    Outputs are fed back as the next iteration's donated output buffers
    (every output element is overwritten), so the loop moves no host data.
    Returns seconds per iteration.
    """
    import time
    import jax
    import numpy as jnp_np
    from jax.sharding import Mesh, PartitionSpec, NamedSharding
    from jax.experimental.shard_map import shard_map
    from concourse import bass2jax
    from concourse.bass2jax import _bass_exec_p, partition_id_tensor, \
        install_neuronx_cc_hook
    import concourse.mybir as mybir

    install_neuronx_cc_hook()
    nc = _get_nc()

    X = np.asarray(inputs["X"], dtype=np.float32)
    WQ = np.asarray(inputs["WQ"], dtype=np.float32)
    WK = np.asarray(inputs["WK"], dtype=np.float32)
    WV = np.asarray(inputs["WV"], dtype=np.float32)
    WO = np.asarray(inputs["WO"], dtype=np.float32)
    pad_mask = np.asarray(inputs["pad_mask"])
    WQT = np.stack([np.concatenate([WQ[2 * p].T, WQ[2 * p + 1].T], axis=1)
                    for p in range(NP)])
    WKT = np.stack([np.concatenate([WK[2 * p].T, WK[2 * p + 1].T], axis=1)
                    for p in range(NP)])
    WVT = np.stack([np.concatenate([WV[4 * g + j].T for j in range(4)], axis=1)
                    for g in range(NQ)])
    WOT = np.ascontiguousarray(WO.T)
    ones_row = np.ones((1, L), np.float32)
    ones_col = np.ones((1, 64), np.float32)
    in_maps = [_prep_core_inputs(X[i], WQT, WKT, WVT, WOT, pad_mask[i],
                                 ones_row, ones_col) for i in range(B)]

    partition_name = (nc.partition_id_tensor.name
                      if nc.partition_id_tensor else None)
    in_names, out_names, out_avals, zero_outs = [], [], [], []
    for alloc in nc.m.functions[0].allocations:
        if not isinstance(alloc, mybir.MemoryLocationSet):
            continue
        name = alloc.memorylocations[0].name
        if alloc.kind == "ExternalInput":
            if name != partition_name:
                in_names.append(name)
        elif alloc.kind == "ExternalOutput":
            out_names.append(name)
            shape = tuple(alloc.tensor_shape)
            dtype = mybir.dt.np(alloc.dtype)
            out_avals.append(jax.core.ShapedArray(shape, dtype))
            zero_outs.append(np.zeros(shape, dtype))
    n_params = len(in_names)
    n_outs = len(out_avals)
    all_in_names = list(in_names) + list(out_names)
    if partition_name is not None:
        all_in_names.append(partition_name)
    donate = tuple(range(n_params, n_params + n_outs))

    def _body(*args):
        operands = list(args)
        if partition_name is not None:
            operands.append(partition_id_tensor())
        outs = _bass_exec_p.bind(
            *operands,
            out_avals=tuple(out_avals),
            in_names=tuple(all_in_names),
            out_names=tuple(out_names),
            lowering_input_output_aliases=(),
            sim_require_finite=True,
            sim_require_nnan=True,
            nc=nc,
        )
        return tuple(outs)

    devices = jax.devices()[:B]
    mesh = Mesh(np.asarray(devices), ("core",))
    in_specs = (PartitionSpec("core"),) * (n_params + n_outs)
    out_specs = (PartitionSpec("core"),) * n_outs
    sharded = jax.jit(
        shard_map(_body, mesh=mesh, in_specs=in_specs,
                  out_specs=out_specs, check_rep=False),
        donate_argnums=donate, keep_unused=True)

    shard = NamedSharding(mesh, PartitionSpec("core"))
    concat_in = [
        jax.device_put(
            np.concatenate([np.asarray(in_maps[c][n]) for c in range(B)],
                           axis=0), shard)
        for n in in_names
    ]
    cur_outs = [
        jax.device_put(
            np.zeros((B * z.shape[0], *z.shape[1:]), z.dtype), shard)
        for z in zero_outs
    ]

    # warmup (also compiles)
    cur_outs = list(sharded(*concat_in, *cur_outs))
    jax.block_until_ready(cur_outs)

    t0 = time.time()
    for _ in range(reps):
        cur_outs = list(sharded(*concat_in, *cur_outs))
    jax.block_until_ready(cur_outs)
    t1 = time.time()
    return (t1 - t0) / reps


# revision 41
# speedup vs baseline: 2.4089x; 2.4089x over previous
"""Self-contained Trainium2 Bass kernel for masked multi-head attention.

Problem: B=8, L=1024, D=1024, H=16, DK=DV=64, fp32, pad-masked softmax.
Returns (out [B,L,D], attn [B,H,L,L]) matching the jax reference.

Strategy: data-parallel over batch B across the 8 NeuronCores. Each core runs
the full attention for one batch element. All matmuls run as float32r (full
PE rate at N>=256). The pad mask is folded in three ways:
  - key mask on S:  augmented contraction row (ones x -240000*padk), K=65
  - query mask on P = exp(S): per-partition bias AP on the ScalarE exp
  - key mask on P^T = exp(S^T): per-partition bias AP on the second exp
Row sums come free via the ScalarE activation accum_out. The second matmul
O^T = V^T P^T uses V as the stationary operand (natural layout from the
V projection) so no transposes of the 16M-element attention matrix are ever
needed; instead S^T is computed directly by swapping the QK matmul operands.
Host-side prep only re-lays-out inputs (transposes of X/W weights).

Per-core schedule (Tile cost model): ~418 us against per-engine busy
floors of ~290 us (ScalarE exps), ~265 us (DMA: 20 MB in + 68 MB out),
~260 us (PE fp32r matmuls). The pair loop software-pipelines weight
loads and Q/K projections one pair ahead (DMA prefetches issued during
the store-free C phase so the attn-store stream keeps HBM saturated
during B), and the O^T accumulation runs one step behind the S^T
production so no in-order PE stream ever parks behind an exp wait.
Accuracy vs a float64 reference: attn ~2.6e-4 rel, out ~2.3e-3 rel
(out passes through the bf16 O^T path; attn is fp32r/fp32 end-to-end).
"""

from contextlib import ExitStack

import os

import numpy as np

import concourse.bass as bass
import concourse.tile as tile
from concourse import mybir
import bass_rust
from concourse.bass_utils import run_bass_kernel_spmd

F32 = mybir.dt.float32
F32R = mybir.dt.float32r
BF16 = mybir.dt.bfloat16
EXP = mybir.ActivationFunctionType.Exp

B, L, D, H, DK, DV = 8, 1024, 1024, 16, 64, 64
NP = 8          # head pairs
NQ = 4          # head quads
NT = 8          # 128-row tiles per L
SCALE = 0.125   # 1/sqrt(DK)
BIGK = -240000.0   # key-mask addend inside S (pre-scale)
BIGQ = -30000.0    # query-mask bias on exp (post-scale)

_wsplit_ctr = [0]


def _split_multi_waits(nc, max_waits=1):
    """This container's walrus only accepts one sync-wait command per
    instruction; hoist extra waits onto NOPs inserted just before."""
    for f in nc.m.functions:
        for bb in f.blocks:
            insts = list(bb.instructions)
            new, changed = [], False
            for inst in insts:
                si = inst.sync_info
                if si is not None:
                    waits = list(si.on_wait)
                    if len(waits) > max_waits:
                        for w in waits[:-max_waits]:
                            _wsplit_ctr[0] += 1
                            nop = mybir.InstNoOp(
                                name=f"I-wsplit-{_wsplit_ctr[0]}", ins=[], outs=[])
                            nop.engine = inst.engine
                            nop.sync_info = bass_rust.SyncInfo(
                                on_wait=[w], on_update=[])
                            new.append(nop)
                        inst.sync_info = bass_rust.SyncInfo(
                            on_wait=waits[-max_waits:],
                            on_update=list(si.on_update))
                        changed = True
                new.append(inst)
            if changed:
                bb.instructions = new


def _r(ap):
    return ap


def build_nc():
    nc = bass.Bass("TRN2", target_bir_lowering=False, debug=False,
                   num_devices=1)

    xt_d = nc.dram_tensor("xt", [D, L], F32R, kind="ExternalInput")
    wqt_d = nc.dram_tensor("wqt", [NP, D, 128], F32R, kind="ExternalInput")
    wkt_d = nc.dram_tensor("wkt", [NP, D, 128], F32R, kind="ExternalInput")
    wvt_d = nc.dram_tensor("wvt", [NQ, D, 256], F32R, kind="ExternalInput")
    wot_d = nc.dram_tensor("wot", [D, D], F32R, kind="ExternalInput")
    qbias_d = nc.dram_tensor("qbias", [128, NT], F32, kind="ExternalInput")
    rmask_d = nc.dram_tensor("rmask", [128, NT], F32, kind="ExternalInput")
    ones_row_d = nc.dram_tensor("ones_row", [1, L], F32R, kind="ExternalInput")
    kmask_row_d = nc.dram_tensor("kmask_row", [1, L], F32R, kind="ExternalInput")
    ones_col_d = nc.dram_tensor("ones_col", [1, 64], F32R, kind="ExternalInput")

    out_d = nc.dram_tensor("out", [L, D], F32, kind="ExternalOutput")
    attn_d = nc.dram_tensor("attn", [H, L, L], F32, kind="ExternalOutput")

    with tile.TileContext(nc) as tc:
        with ExitStack() as stack:
            ent = stack.enter_context
            # ---------- persistent pools (live whole kernel) ----------
            catt_pool = ent(tc.tile_pool(name="catt", bufs=NP))
            small_pool = ent(tc.tile_pool(name="small", bufs=1))
            rsum_pool = ent(tc.tile_pool(name="rsum", bufs=2))
            rflat_pool = ent(tc.tile_pool(name="rflat", bufs=1))
            qbias_t = small_pool.tile([128, NT], F32)
            nc.sync.dma_start(qbias_t[:], qbias_d[:])
            rmask_t = small_pool.tile([128, NT], F32)
            nc.sync.dma_start(rmask_t[:], rmask_d[:])
            ones_row_t = small_pool.tile([1, L], F32R)
            nc.sync.dma_start(ones_row_t[:], ones_row_d[:])
            kmask_row_t = small_pool.tile([1, L], F32R)
            nc.sync.dma_start(kmask_row_t[:], kmask_row_d[:])
            ones_col_t = small_pool.tile([1, 64], F32R)
            nc.sync.dma_start(ones_col_t[:], ones_col_d[:])

            warm = small_pool.tile([128, NT], F32, name="warm")
            nc.scalar.activation(warm[:], qbias_t[:], EXP, bias=0.0,
                                 scale=0.0)


            catt = []  # O^T (normalized) per pair: [128 (hv), L(q)]
            for p in range(NP):
                catt.append(catt_pool.tile([128, L], F32R, name=f"catt{p}", tag=f"catt{p}", bufs=1))

            with ExitStack() as pair_stack:
                pent = pair_stack.enter_context
                # ---------- pair-phase pools ----------
                xt_pool = pent(tc.tile_pool(name="xt", bufs=1))
                wq_pool = pent(tc.tile_pool(name="wq", bufs=2))
                wk_pool = pent(tc.tile_pool(name="wk", bufs=2))
                wvt_pool = pent(tc.tile_pool(name="wvt", bufs=2))
                vsb_pool = pent(tc.tile_pool(name="vsb", bufs=2))
                qaug_pool = pent(tc.tile_pool(name="qaug", bufs=2))
                kaug_pool = pent(tc.tile_pool(name="kaug", bufs=2))
                p_pool = pent(tc.tile_pool(name="pp", bufs=8))
                pt_pool = pent(tc.tile_pool(name="pt", bufs=3))
                rr_pool = pent(tc.tile_pool(name="rr", bufs=1))
                ps_m = pent(tc.tile_pool(name="ps_m", bufs=3, space="PSUM"))
                ps_o = pent(tc.tile_pool(name="ps_o", bufs=1, space="PSUM"))
                xt_big = xt_pool.tile([128, NT * L], F32R, name="xt_big",
                                      tag="xt")
                nc.sync.dma_start(
                    xt_big[:],
                    xt_d.rearrange("(c p) l -> p c l", p=128))
                xt = [xt_big[:, c * L:(c + 1) * L] for c in range(NT)]

                def v_parts(g):
                    """V projection for quad g, split into a load closure
                    (issued during the preceding C phase) and per-st matmul
                    fillers (run during the odd pair's B phase)."""
                    res = {"vsb": []}

                    def loads():
                        wvt_big = wvt_pool.tile(
                            [128, NT * 256], F32R, name=f"wvt{g}", tag="wvt")
                        nc.gpsimd.dma_start(
                            wvt_big[:],
                            wvt_d[g].rearrange("(c p) j -> p c j", p=128))
                        res["wvt"] = wvt_big

                    def mk(st):
                        def run():
                            wvt_big = res["wvt"]
                            v_ps = ps_m.tile([128, L], F32,
                                             name=f"vps{g}_{st}",
                                             tag="m")[:, 0:256]
                            for c in range(NT):
                                nc.tensor.matmul(
                                    v_ps[:],
                                    _r(xt[c][:, st * 128:(st + 1) * 128]),
                                    _r(wvt_big[:, c * 256:(c + 1) * 256]),
                                    start=(c == 0), stop=(c == NT - 1))
                            v_t = vsb_pool.tile([128, 256], BF16,
                                                name=f"vsb{g}_{st}",
                                                tag=f"vsb{st}")
                            nc.vector.tensor_copy(v_t[:], v_ps[:])
                            res["vsb"].append(v_t)
                        return run
                    return loads, [mk(st) for st in range(NT)], res

                def qk_parts(p):
                    res = {}

                    def loads():
                        res["wq"] = _project_load(nc, p, wq_pool, wqt_d, "q")
                        res["wk"] = _project_load(nc, p, wk_pool, wkt_d, "k")

                    def mm_q():
                        res["q"] = _project(nc, p, xt, qaug_pool, res["wq"],
                                            ones_row_t[:, :], ps_m, "q")

                    def mm_k():
                        res["k"] = _project(nc, p, xt, kaug_pool, res["wk"],
                                            kmask_row_t[:, :], ps_m, "k")
                    return loads, [mm_q, mm_k], res

                # spin the PE while input DMAs stream so the HAM clock
                # gate reaches 2.4 GHz before the first projection
                warm_ps = ps_m.tile([64, L], F32, name="warm_ps", tag="m")
                for w in range(24):
                    nc.tensor.matmul(warm_ps[:, 0:512],
                                     _r(ones_col_t[:, :]),
                                     _r(ones_row_t[:, 0:512]),
                                     start=True, stop=True)

                # prologue: pair 0 + quad 0 eagerly, pair 1 loads eagerly
                l0, m0, r0 = qk_parts(0)
                l0()
                for m in m0:
                    m()
                qk = (r0["q"], r0["k"])
                vl0, vm0, vr0 = v_parts(0)
                vl0()
                for m in vm0:
                    m()
                vsb_cur = vr0["vsb"]

                nxt_qk = None
                if NP > 1:
                    l1, m1, r1 = qk_parts(1)
                    l1()
                    nxt_qk = (m1, r1)
                nxt_v = None

                deferred = []
                for p in range(NP):
                    g = p // 2
                    fillers = list(deferred)
                    preloads = []
                    if nxt_qk is not None:
                        fillers += nxt_qk[0]
                    if p % 2 == 1 and nxt_v is not None:
                        fillers += nxt_v[0]
                    # loads for work whose matmul fillers run next pair
                    nxt2_qk = None
                    if p + 2 < NP:
                        l2, m2, r2 = qk_parts(p + 2)
                        preloads.append(l2)
                        nxt2_qk = (m2, r2)
                    if p % 2 == 0 and g + 1 < NQ:
                        vl, vm, vr = v_parts(g + 1)
                        preloads.append(vl)
                        nxt_v = (vm, vr)
                    deferred = _attend_pair(
                        nc, p, qk, vsb_cur,
                        p_pool, pt_pool, rr_pool,
                        rsum_pool, rflat_pool, ps_m, ps_o,
                        attn_d, qbias_t, rmask_t, ones_col_t,
                        catt[p], fillers, preloads)
                    while fillers:
                        fillers.pop(0)()
                    if nxt_qk is not None:
                        qk = (nxt_qk[1]["q"], nxt_qk[1]["k"])
                    nxt_qk = nxt2_qk
                    if p % 2 == 1 and nxt_v is not None:
                        vsb_cur = nxt_v[1]["vsb"]
                for f in deferred:
                    f()

            # ---------- final output projection ----------
            with ExitStack() as fin_stack:
                fent = fin_stack.enter_context
                wot_pool = fent(tc.tile_pool(name="wot", bufs=1))
                outsb_pool = fent(tc.tile_pool(name="outsb", bufs=2))
                ps_f = fent(tc.tile_pool(name="ps_f", bufs=2, space="PSUM"))
                wot_big = wot_pool.tile([128, NP * D], F32R, name="wot_big",
                                        tag="wot")
                wot_r = wot_d.rearrange("(c p) l -> p c l", p=128)
                nc.sync.dma_start(wot_big[:, 0:4 * D], wot_r[:, 0:4, :])
                nc.sync.dma_start(wot_big[:, 4 * D:8 * D], wot_r[:, 4:8, :])
                wot = [wot_big[:, p * D:(p + 1) * D] for p in range(NP)]
                for qt in range(NT):
                    o_ps = ps_f.tile([128, D], F32)
                    for dh in range(2):
                        for p in range(NP):
                            nc.tensor.matmul(
                                o_ps[:, dh * 512:(dh + 1) * 512],
                                _r(catt[p][:, qt * 128:(qt + 1) * 128]),
                                _r(wot[p][:, dh * 512:(dh + 1) * 512]),
                                start=(p == 0), stop=(p == NP - 1))
                    o_sb = outsb_pool.tile([128, D], F32)
                    nc.vector.tensor_copy(o_sb[:], o_ps[:])
                    nc.gpsimd.dma_start(
                        out_d[qt * 128:(qt + 1) * 128, :], o_sb[:])

    _split_multi_waits(nc)
    return nc


def _project_load(nc, p, w_pool, w_dram, kind):
    w_big = w_pool.tile([128, NT * 128], F32R, name=f"w{kind}{p}",
                        tag=f"w{kind}")
    nc.gpsimd.dma_start(
        w_big[:], w_dram[p].rearrange("(c q) j -> q c j", q=128))
    return w_big


def _project(nc, p, xt, aug_pool, w_big, mask_row, ps_m, kind):
    """Q^T/K^T pair projection -> two augmented per-head tiles [65, L]
    (rows 0-63: head data, row 64: ones / -BIGK*padk)."""
    wts = [w_big[:, c * 128:(c + 1) * 128] for c in range(NT)]
    pr_ps = ps_m.tile([128, L], F32, name=f"prps{kind}{p}", tag="m")
    for lh in range(2):
        for c in range(NT):
            nc.tensor.matmul(
                pr_ps[:, lh * 512:(lh + 1) * 512],
                _r(wts[c][:]),
                _r(xt[c][:, lh * 512:(lh + 1) * 512]),
                start=(c == 0), stop=(c == NT - 1))
    augs = []
    for hi in range(2):
        aug = aug_pool.tile([65, L], F32R, name=f"aug{kind}{p}_{hi}",
                            tag=f"aug{hi}")
        nc.vector.tensor_copy(aug[0:64, :],
                              pr_ps[hi * 64:(hi + 1) * 64, :])
        nc.gpsimd.dma_start(aug[64:65, :], mask_row)
        augs.append(aug)
    return augs


def _attend_pair(nc, p, qk, vsb,
                 p_pool, pt_pool, rr_pool, rsum_pool, rflat_pool,
                 ps_m, ps_o, attn_d, qbias_t, rmask_t, ones_col_t, catt_p,
                 fillers=(), preloads=()):
    fillers = list(fillers) if not isinstance(fillers, list) else fillers
    qaug, kaug = qk
    h0, h1 = 2 * p, 2 * p + 1

    rsum = [rsum_pool.tile([128, NT], F32, name=f"rsum{p}_{i}", tag=f"rsum{i}")
            for i in range(2)]
    rstar = [rsum_pool.tile([128, NT], F32, name=f"rstar{p}_{i}",
                            tag=f"rstar{i}")
             for i in range(2)]
    rstar_r = [rsum_pool.tile([128, NT], F32R, name=f"rstarr{p}_{i}",
                              tag=f"rstarr{i}")
               for i in range(2)]

    # ---- S -> exp -> A -> store (B phase) ----
    for hi in range(2):
        h = h0 if hi == 0 else h1
        for qt in range(NT):
            s_ps = ps_m.tile([128, L], F32, tag="m")
            for sh in range(2):
                nc.tensor.matmul(
                    s_ps[:, sh * 512:(sh + 1) * 512],
                    _r(qaug[hi][:, qt * 128:(qt + 1) * 128]),
                    _r(kaug[hi][:, sh * 512:(sh + 1) * 512]),
                    start=True, stop=True)
            p_t = p_pool.tile([128, L], F32)
            nc.scalar.activation(p_t[:], s_ps[:], EXP,
                                 bias=qbias_t[:, qt:qt + 1], scale=SCALE,
                                 accum_out=rsum[hi][:, qt:qt + 1])
            nc.vector.tensor_scalar_add(rstar[hi][:, qt:qt + 1],
                                        rsum[hi][:, qt:qt + 1], 1e-12)
            nc.vector.reciprocal(rstar[hi][:, qt:qt + 1],
                                 rstar[hi][:, qt:qt + 1])
            nc.vector.tensor_mul(rstar[hi][:, qt:qt + 1],
                                 rstar[hi][:, qt:qt + 1],
                                 rmask_t[:, qt:qt + 1])
            nc.vector.tensor_scalar_mul(p_t[:], p_t[:],
                                        rstar[hi][:, qt:qt + 1])
            nc.sync.dma_start(attn_d[h, qt * 128:(qt + 1) * 128, :],
                              p_t[:])
            if fillers and (hi, qt) >= (0, 2):
                fillers.pop(0)()

    # C phase start: issue prefetch DMAs for the next pair/quad plus the
    # r-flatten DMAs of this pair (the B phase saturates HBM with stores).
    rflat = [rflat_pool.tile([1, L], F32R, name=f"rflat{p}_{hi2}",
                             tag=f"rflat{hi2}")
             for hi2 in range(2)]

    def d_loads():
        for hi2 in range(2):
            nc.vector.tensor_copy(rstar_r[hi2][:], rstar[hi2][:])
            for t in range(NT):
                nc.gpsimd.dma_start(rflat[hi2][:, t * 128:(t + 1) * 128],
                                    rstar_r[hi2][:, t:t + 1])
    for f in preloads:
        f()
    d_loads()

    # ---- S^T -> exp -> P^T ; O^T accumulation (C phase; bf16 O^T packs
    # both heads into one tile via column tile_position). The O^T matmuls
    # for step st are emitted AFTER the S^T matmuls of st+1: engines issue
    # in order, so an O^T waiting on exp2(st) must not sit ahead of ready
    # S^T work in the PE stream. ----
    ot_ps = ps_o.tile([128, L], F32, name=f"ot{p}", tag="ot")
    pt_hist = {}

    def st_mms(st):
        st_ps = [ps_m.tile([128, L], F32, name=f"stps{p}_{st}_{i}", tag="m")
                 for i in range(2)]
        for hi in range(2):
            for qh in range(2):
                nc.tensor.matmul(
                    st_ps[hi][:, qh * 512:(qh + 1) * 512],
                    _r(kaug[hi][:, st * 128:(st + 1) * 128]),
                    _r(qaug[hi][:, qh * 512:(qh + 1) * 512]),
                    start=True, stop=True)
        pt_t = [pt_pool.tile([128, L], BF16, name=f"pt{p}_{st}_{i}",
                             tag=f"pt{i}")
                for i in range(2)]
        for hi in range(2):
            nc.scalar.activation(pt_t[hi][:], st_ps[hi][:], EXP,
                                 bias=0.0, scale=SCALE)
        pt_hist[st] = pt_t

    def ot_mms(st):
        pt_t = pt_hist.pop(st)
        for qh in range(2):
            for hi in range(2):
                nc.tensor.matmul(
                    ot_ps[hi * 64:(hi + 1) * 64, qh * 512:(qh + 1) * 512],
                    vsb[st][:, ((p % 2) * 2 + hi) * 64:
                            ((p % 2) * 2 + hi + 1) * 64],
                    pt_t[hi][:, qh * 512:(qh + 1) * 512],
                    start=(st == 0), stop=(st == NT - 1),
                    tile_position=(0, hi * 64))

    for st in range(NT):
        st_mms(st)
        if st >= 1:
            ot_mms(st - 1)

    # ---- normalize O^T by r (replicated along partitions). Deferred:
    # returned as closures the caller interleaves into the NEXT pair's B
    # phase, keeping this off the inter-pair critical path. ----
    def d_phase(hi):
        def run():
            rr_ps = ps_m.tile([64, L], F32, name=f"rrps{p}_{hi}", tag="m")
            for qh in range(2):
                nc.tensor.matmul(
                    rr_ps[:, qh * 512:(qh + 1) * 512],
                    _r(ones_col_t[:, :]),
                    _r(rflat[hi][:, qh * 512:(qh + 1) * 512]),
                    start=True, stop=True)
            rr_sb = rr_pool.tile([64, L], F32, name=f"rrsb{p}_{hi}",
                                 tag=f"rrsb{hi}")
            nc.vector.tensor_copy(rr_sb[:], rr_ps[:])
            nc.vector.tensor_mul(catt_p[hi * 64:(hi + 1) * 64, :],
                                 ot_ps[hi * 64:(hi + 1) * 64, :], rr_sb[:])
        return run

    def ot_tail():
        ot_mms(NT - 1)
    return [ot_tail, d_phase(0), d_phase(1)]


_CACHED = {}
_LAST_EXEC_NS = None
_LAST_RES = None


def _get_nc():
    if "nc" not in _CACHED:
        _CACHED["nc"] = build_nc()
    return _CACHED["nc"]


def _prep_core_inputs(Xi, WQT, WKT, WVT, WOT, pmi,
                      ones_row, ones_col):
    pm = pmi.astype(np.float32)
    pm_tiled = pm.reshape(NT, 128).T.copy()      # [128, NT]
    return {
        "xt": np.ascontiguousarray(Xi.T),
        "wqt": WQT, "wkt": WKT, "wvt": WVT, "wot": WOT,
        "qbias": np.ascontiguousarray(BIGQ * pm_tiled),
        "rmask": np.ascontiguousarray(1.0 - pm_tiled),
        "ones_row": ones_row,
        "kmask_row": np.ascontiguousarray((BIGK * pm)[None, :]),
        "ones_col": ones_col,
    }


def kernel(X, WQ, WK, WV, WO, pad_mask):
    X = np.asarray(X, dtype=np.float32)
    WQ = np.asarray(WQ, dtype=np.float32)
    WK = np.asarray(WK, dtype=np.float32)
    WV = np.asarray(WV, dtype=np.float32)
    WO = np.asarray(WO, dtype=np.float32)
    pad_mask = np.asarray(pad_mask)

    WQT = np.stack([np.concatenate([WQ[2 * p].T, WQ[2 * p + 1].T], axis=1)
                    for p in range(NP)])          # [NP, D, 128]
    WKT = np.stack([np.concatenate([WK[2 * p].T, WK[2 * p + 1].T], axis=1)
                    for p in range(NP)])
    WVT = np.stack([np.concatenate([WV[4 * g + j].T for j in range(4)], axis=1)
                    for g in range(NQ)])          # [NQ, D, 256]
    WOT = np.ascontiguousarray(WO.T)
    ones_row = np.ones((1, L), np.float32)
    ones_col = np.ones((1, 64), np.float32)

    in_maps = [
        _prep_core_inputs(X[i], WQT, WKT, WVT, WOT, pad_mask[i],
                          ones_row, ones_col)
        for i in range(B)
    ]

    nc = _get_nc()
    trace = bool(os.environ.get("KERNEL_TRACE"))
    kw = {}
    if trace:
        kw["trace"] = True
        kw["tmpdir"] = os.environ.get("KERNEL_TRACE_DIR") or None
    res = run_bass_kernel_spmd(nc, in_maps, list(range(B)), **kw)
    global _LAST_EXEC_NS, _LAST_RES
    _LAST_RES = res
    _LAST_EXEC_NS = res.exec_time_ns

    out = np.stack([res.results[i]["out"] for i in range(B)])
    attn = np.stack([res.results[i]["attn"] for i in range(B)])
    return out, attn


if __name__ == "__main__":
    rng = np.random.default_rng(0)
    X = rng.standard_normal((B, L, D), dtype=np.float32)
    WQ = (rng.standard_normal((H, DK, D), dtype=np.float32) * 0.02)
    WK = (rng.standard_normal((H, DK, D), dtype=np.float32) * 0.02)
    WV = (rng.standard_normal((H, DV, D), dtype=np.float32) * 0.02)
    WO = (rng.standard_normal((D, H * DV), dtype=np.float32) * 0.02)
    pm = rng.integers(0, 2, size=(B, L)).astype(bool)
    out, attn = kernel(X=X, WQ=WQ, WK=WK, WV=WV, WO=WO, pad_mask=pm)
    print("out", out.shape, "attn", attn.shape)


def bench(reps=16, **inputs):
    """Time repeated on-device executions with resident inputs.

    Outputs are fed back as the next iteration's donated output buffers
    (every output element is overwritten), so the loop moves no host data.
    Returns seconds per iteration.
    """
    import time
    import jax
    import numpy as jnp_np
    from jax.sharding import Mesh, PartitionSpec, NamedSharding
    from jax.experimental.shard_map import shard_map
    from concourse import bass2jax
    from concourse.bass2jax import _bass_exec_p, partition_id_tensor, \
        install_neuronx_cc_hook
    import concourse.mybir as mybir

    install_neuronx_cc_hook()
    nc = _get_nc()

    X = np.asarray(inputs["X"], dtype=np.float32)
    WQ = np.asarray(inputs["WQ"], dtype=np.float32)
    WK = np.asarray(inputs["WK"], dtype=np.float32)
    WV = np.asarray(inputs["WV"], dtype=np.float32)
    WO = np.asarray(inputs["WO"], dtype=np.float32)
    pad_mask = np.asarray(inputs["pad_mask"])
    WQT = np.stack([np.concatenate([WQ[2 * p].T, WQ[2 * p + 1].T], axis=1)
                    for p in range(NP)])
    WKT = np.stack([np.concatenate([WK[2 * p].T, WK[2 * p + 1].T], axis=1)
                    for p in range(NP)])
    WVT = np.stack([np.concatenate([WV[4 * g + j].T for j in range(4)], axis=1)
                    for g in range(NQ)])
    WOT = np.ascontiguousarray(WO.T)
    ones_row = np.ones((1, L), np.float32)
    ones_col = np.ones((1, 64), np.float32)
    in_maps = [_prep_core_inputs(X[i], WQT, WKT, WVT, WOT, pad_mask[i],
                                 ones_row, ones_col) for i in range(B)]

    partition_name = (nc.partition_id_tensor.name
                      if nc.partition_id_tensor else None)
    in_names, out_names, out_avals, zero_outs = [], [], [], []
    for alloc in nc.m.functions[0].allocations:
        if not isinstance(alloc, mybir.MemoryLocationSet):
            continue
        name = alloc.memorylocations[0].name
        if alloc.kind == "ExternalInput":
            if name != partition_name:
                in_names.append(name)
        elif alloc.kind == "ExternalOutput":
            out_names.append(name)
            shape = tuple(alloc.tensor_shape)
            dtype = mybir.dt.np(alloc.dtype)
            out_avals.append(jax.core.ShapedArray(shape, dtype))
            zero_outs.append(np.zeros(shape, dtype))
    n_params = len(in_names)
    n_outs = len(out_avals)
    all_in_names = list(in_names) + list(out_names)
    if partition_name is not None:
        all_in_names.append(partition_name)
    donate = tuple(range(n_params, n_params + n_outs))

    def _body(*args):
        operands = list(args)
        if partition_name is not None:
            operands.append(partition_id_tensor())
        outs = _bass_exec_p.bind(
            *operands,
            out_avals=tuple(out_avals),
            in_names=tuple(all_in_names),
            out_names=tuple(out_names),
            lowering_input_output_aliases=(),
            sim_require_finite=True,
            sim_require_nnan=True,
            nc=nc,
        )
        return tuple(outs)

    devices = jax.devices()[:B]
    mesh = Mesh(np.asarray(devices), ("core",))
    in_specs = (PartitionSpec("core"),) * (n_params + n_outs)
    out_specs = (PartitionSpec("core"),) * n_outs
    sharded = jax.jit(
        shard_map(_body, mesh=mesh, in_specs=in_specs,
                  out_specs=out_specs, check_rep=False),
        donate_argnums=donate, keep_unused=True)

    shard = NamedSharding(mesh, PartitionSpec("core"))
    concat_in = [
        jax.device_put(
            np.concatenate([np.asarray(in_maps[c][n]) for c in range(B)],
                           axis=0), shard)
        for n in in_names
    ]
    cur_outs = [
        jax.device_put(
            np.zeros((B * z.shape[0], *z.shape[1:]), z.dtype), shard)
        for z in zero_outs
    ]

    # warmup (also compiles)
    cur_outs = list(sharded(*concat_in, *cur_outs))
    jax.block_until_ready(cur_outs)

    t0 = time.time()
    for _ in range(reps):
        cur_outs = list(sharded(*concat_in, *cur_outs))
    jax.block_until_ready(cur_outs)
    t1 = time.time()
    return (t1 - t0) / reps


# revision 48
# speedup vs baseline: 13.8070x; 5.7317x over previous
"""Self-contained Trainium2 Bass kernel for masked multi-head attention.

Problem: B=8, L=1024, D=1024, H=16, DK=DV=64, fp32, pad-masked softmax.
Returns (out [B,L,D], attn [B,H,L,L]) matching the jax reference.

Strategy: data-parallel over batch B across the 8 NeuronCores. Each core runs
the full attention for one batch element. All matmuls run as float32r (full
PE rate at N>=256). The pad mask is folded in three ways:
  - key mask on S:  augmented contraction row (ones x -240000*padk), K=65
  - query mask on P = exp(S): per-partition bias AP on the ScalarE exp
  - key mask on P^T = exp(S^T): per-partition bias AP on the second exp
Row sums come free via the ScalarE activation accum_out. The second matmul
O^T = V^T P^T uses V as the stationary operand (natural layout from the
V projection) so no transposes of the 16M-element attention matrix are ever
needed; instead S^T is computed directly by swapping the QK matmul operands.
Host-side prep only re-lays-out inputs (transposes of X/W weights).

Per-core schedule (Tile cost model): ~418 us against per-engine busy
floors of ~290 us (ScalarE exps), ~265 us (DMA: 20 MB in + 68 MB out),
~260 us (PE fp32r matmuls). The pair loop software-pipelines weight
loads and Q/K projections one pair ahead (DMA prefetches issued during
the store-free C phase so the attn-store stream keeps HBM saturated
during B), and the O^T accumulation runs one step behind the S^T
production so no in-order PE stream ever parks behind an exp wait.
Accuracy vs a float64 reference: attn ~2.6e-4 rel, out ~2.3e-3 rel
(out passes through the bf16 O^T path; attn is fp32r/fp32 end-to-end).
"""

from contextlib import ExitStack

import os

import numpy as np

import concourse.bass as bass
import concourse.tile as tile
from concourse import mybir
import bass_rust
from concourse.bass_utils import run_bass_kernel_spmd

F32 = mybir.dt.float32
F32R = mybir.dt.float32r
BF16 = mybir.dt.bfloat16
EXP = mybir.ActivationFunctionType.Exp

B, L, D, H, DK, DV = 8, 1024, 1024, 16, 64, 64
NP = 8          # head pairs
NQ = 4          # head quads
NT = 8          # 128-row tiles per L
SCALE = 0.125   # 1/sqrt(DK)
BIGK = -240000.0   # key-mask addend inside S (pre-scale)
BIGQ = -30000.0    # query-mask bias on exp (post-scale)

_wsplit_ctr = [0]


def _split_multi_waits(nc, max_waits=1):
    """This container's walrus only accepts one sync-wait command per
    instruction; hoist extra waits onto NOPs inserted just before."""
    for f in nc.m.functions:
        for bb in f.blocks:
            insts = list(bb.instructions)
            new, changed = [], False
            for inst in insts:
                si = inst.sync_info
                if si is not None:
                    waits = list(si.on_wait)
                    if len(waits) > max_waits:
                        for w in waits[:-max_waits]:
                            _wsplit_ctr[0] += 1
                            nop = mybir.InstNoOp(
                                name=f"I-wsplit-{_wsplit_ctr[0]}", ins=[], outs=[])
                            nop.engine = inst.engine
                            nop.sync_info = bass_rust.SyncInfo(
                                on_wait=[w], on_update=[])
                            new.append(nop)
                        inst.sync_info = bass_rust.SyncInfo(
                            on_wait=waits[-max_waits:],
                            on_update=list(si.on_update))
                        changed = True
                new.append(inst)
            if changed:
                bb.instructions = new


def _r(ap):
    return ap


def build_nc():
    nc = bass.Bass("TRN2", target_bir_lowering=False, debug=False,
                   num_devices=1)

    xt_d = nc.dram_tensor("xt", [D, L], F32R, kind="ExternalInput")
    wqt_d = nc.dram_tensor("wqt", [NP, D, 128], F32R, kind="ExternalInput")
    wkt_d = nc.dram_tensor("wkt", [NP, D, 128], F32R, kind="ExternalInput")
    wvt_d = nc.dram_tensor("wvt", [NQ, D, 256], F32R, kind="ExternalInput")
    wot_d = nc.dram_tensor("wot", [D, D], F32R, kind="ExternalInput")
    qbias_d = nc.dram_tensor("qbias", [128, NT], F32, kind="ExternalInput")
    rmask_d = nc.dram_tensor("rmask", [128, NT], F32, kind="ExternalInput")
    ones_row_d = nc.dram_tensor("ones_row", [1, L], F32R, kind="ExternalInput")
    kmask_row_d = nc.dram_tensor("kmask_row", [1, L], F32R, kind="ExternalInput")
    ones_col_d = nc.dram_tensor("ones_col", [1, 64], F32R, kind="ExternalInput")

    out_d = nc.dram_tensor("out", [L, D], F32, kind="ExternalOutput")
    attn_d = nc.dram_tensor("attn", [H, L, L], F32, kind="ExternalOutput")

    with tile.TileContext(nc) as tc:
        with ExitStack() as stack:
            ent = stack.enter_context
            # ---------- persistent pools (live whole kernel) ----------
            catt_pool = ent(tc.tile_pool(name="catt", bufs=NP))
            small_pool = ent(tc.tile_pool(name="small", bufs=1))
            rsum_pool = ent(tc.tile_pool(name="rsum", bufs=2))
            rflat_pool = ent(tc.tile_pool(name="rflat", bufs=1))
            qbias_t = small_pool.tile([128, NT], F32)
            nc.sync.dma_start(qbias_t[:], qbias_d[:])
            rmask_t = small_pool.tile([128, NT], F32)
            nc.sync.dma_start(rmask_t[:], rmask_d[:])
            ones_row_t = small_pool.tile([1, L], F32R)
            nc.sync.dma_start(ones_row_t[:], ones_row_d[:])
            kmask_row_t = small_pool.tile([1, L], F32R)
            nc.sync.dma_start(kmask_row_t[:], kmask_row_d[:])
            ones_col_t = small_pool.tile([1, 64], F32R)
            nc.sync.dma_start(ones_col_t[:], ones_col_d[:])

            warm = small_pool.tile([128, NT], F32, name="warm")
            nc.scalar.activation(warm[:], qbias_t[:], EXP, bias=0.0,
                                 scale=0.0)


            catt = []  # O^T (normalized) per pair: [128 (hv), L(q)]
            for p in range(NP):
                catt.append(catt_pool.tile([128, L], F32R, name=f"catt{p}", tag=f"catt{p}", bufs=1))

            with ExitStack() as pair_stack:
                pent = pair_stack.enter_context
                # ---------- pair-phase pools ----------
                xt_pool = pent(tc.tile_pool(name="xt", bufs=1))
                wq_pool = pent(tc.tile_pool(name="wq", bufs=2))
                wk_pool = pent(tc.tile_pool(name="wk", bufs=2))
                wvt_pool = pent(tc.tile_pool(name="wvt", bufs=2))
                vsb_pool = pent(tc.tile_pool(name="vsb", bufs=2))
                qaug_pool = pent(tc.tile_pool(name="qaug", bufs=2))
                kaug_pool = pent(tc.tile_pool(name="kaug", bufs=2))
                p_pool = pent(tc.tile_pool(name="pp", bufs=8))
                pt_pool = pent(tc.tile_pool(name="pt", bufs=3))
                rr_pool = pent(tc.tile_pool(name="rr", bufs=1))
                ps_m = pent(tc.tile_pool(name="ps_m", bufs=3, space="PSUM"))
                ps_o = pent(tc.tile_pool(name="ps_o", bufs=1, space="PSUM"))
                xt_big = xt_pool.tile([128, NT * L], F32R, name="xt_big",
                                      tag="xt")
                nc.sync.dma_start(
                    xt_big[:],
                    xt_d.rearrange("(c p) l -> p c l", p=128))
                xt = [xt_big[:, c * L:(c + 1) * L] for c in range(NT)]

                def v_parts(g):
                    """V projection for quad g, split into a load closure
                    (issued during the preceding C phase) and per-st matmul
                    fillers (run during the odd pair's B phase)."""
                    res = {"vsb": []}

                    def loads():
                        wvt_big = wvt_pool.tile(
                            [128, NT * 256], F32R, name=f"wvt{g}", tag="wvt")
                        nc.gpsimd.dma_start(
                            wvt_big[:],
                            wvt_d[g].rearrange("(c p) j -> p c j", p=128))
                        res["wvt"] = wvt_big

                    def mk(st):
                        def run():
                            wvt_big = res["wvt"]
                            v_ps = ps_m.tile([128, L], F32,
                                             name=f"vps{g}_{st}",
                                             tag="m")[:, 0:256]
                            for c in range(NT):
                                nc.tensor.matmul(
                                    v_ps[:],
                                    _r(xt[c][:, st * 128:(st + 1) * 128]),
                                    _r(wvt_big[:, c * 256:(c + 1) * 256]),
                                    start=(c == 0), stop=(c == NT - 1))
                            v_t = vsb_pool.tile([128, 256], BF16,
                                                name=f"vsb{g}_{st}",
                                                tag=f"vsb{st}")
                            nc.vector.tensor_copy(v_t[:], v_ps[:])
                            res["vsb"].append(v_t)
                        return run
                    return loads, [mk(st) for st in range(NT)], res

                def qk_parts(p):
                    res = {}

                    def loads():
                        res["wq"] = _project_load(nc, p, wq_pool, wqt_d, "q")
                        res["wk"] = _project_load(nc, p, wk_pool, wkt_d, "k")

                    def mm_q():
                        res["q"] = _project(nc, p, xt, qaug_pool, res["wq"],
                                            ones_row_t[:, :], ps_m, "q")

                    def mm_k():
                        res["k"] = _project(nc, p, xt, kaug_pool, res["wk"],
                                            kmask_row_t[:, :], ps_m, "k")
                    return loads, [mm_q, mm_k], res

                # spin the PE while input DMAs stream so the HAM clock
                # gate reaches 2.4 GHz before the first projection
                warm_ps = ps_m.tile([64, L], F32, name="warm_ps", tag="m")
                for w in range(24):
                    nc.tensor.matmul(warm_ps[:, 0:512],
                                     _r(ones_col_t[:, :]),
                                     _r(ones_row_t[:, 0:512]),
                                     start=True, stop=True)

                # prologue: pair 0 + quad 0 eagerly, pair 1 loads eagerly
                l0, m0, r0 = qk_parts(0)
                l0()
                for m in m0:
                    m()
                qk = (r0["q"], r0["k"])
                vl0, vm0, vr0 = v_parts(0)
                vl0()
                for m in vm0:
                    m()
                vsb_cur = vr0["vsb"]

                nxt_qk = None
                if NP > 1:
                    l1, m1, r1 = qk_parts(1)
                    l1()
                    nxt_qk = (m1, r1)
                nxt_v = None

                deferred = []
                for p in range(NP):
                    g = p // 2
                    fillers = list(deferred)
                    preloads = []
                    if nxt_qk is not None:
                        fillers += nxt_qk[0]
                    if p % 2 == 1 and nxt_v is not None:
                        fillers += nxt_v[0]
                    # loads for work whose matmul fillers run next pair
                    nxt2_qk = None
                    if p + 2 < NP:
                        l2, m2, r2 = qk_parts(p + 2)
                        preloads.append(l2)
                        nxt2_qk = (m2, r2)
                    if p % 2 == 0 and g + 1 < NQ:
                        vl, vm, vr = v_parts(g + 1)
                        preloads.append(vl)
                        nxt_v = (vm, vr)
                    deferred = _attend_pair(
                        nc, p, qk, vsb_cur,
                        p_pool, pt_pool, rr_pool,
                        rsum_pool, rflat_pool, ps_m, ps_o,
                        attn_d, qbias_t, rmask_t, ones_col_t,
                        catt[p], fillers, preloads)
                    while fillers:
                        fillers.pop(0)()
                    if nxt_qk is not None:
                        qk = (nxt_qk[1]["q"], nxt_qk[1]["k"])
                    nxt_qk = nxt2_qk
                    if p % 2 == 1 and nxt_v is not None:
                        vsb_cur = nxt_v[1]["vsb"]
                for f in deferred:
                    f()

            # ---------- final output projection ----------
            with ExitStack() as fin_stack:
                fent = fin_stack.enter_context
                wot_pool = fent(tc.tile_pool(name="wot", bufs=1))
                outsb_pool = fent(tc.tile_pool(name="outsb", bufs=2))
                ps_f = fent(tc.tile_pool(name="ps_f", bufs=2, space="PSUM"))
                wot_big = wot_pool.tile([128, NP * D], F32R, name="wot_big",
                                        tag="wot")
                wot_r = wot_d.rearrange("(c p) l -> p c l", p=128)
                nc.sync.dma_start(wot_big[:, 0:4 * D], wot_r[:, 0:4, :])
                nc.sync.dma_start(wot_big[:, 4 * D:8 * D], wot_r[:, 4:8, :])
                wot = [wot_big[:, p * D:(p + 1) * D] for p in range(NP)]
                for qt in range(NT):
                    o_ps = ps_f.tile([128, D], F32)
                    for dh in range(2):
                        for p in range(NP):
                            nc.tensor.matmul(
                                o_ps[:, dh * 512:(dh + 1) * 512],
                                _r(catt[p][:, qt * 128:(qt + 1) * 128]),
                                _r(wot[p][:, dh * 512:(dh + 1) * 512]),
                                start=(p == 0), stop=(p == NP - 1))
                    o_sb = outsb_pool.tile([128, D], F32)
                    nc.vector.tensor_copy(o_sb[:], o_ps[:])
                    nc.gpsimd.dma_start(
                        out_d[qt * 128:(qt + 1) * 128, :], o_sb[:])

    _split_multi_waits(nc)
    return nc


def _project_load(nc, p, w_pool, w_dram, kind):
    w_big = w_pool.tile([128, NT * 128], F32R, name=f"w{kind}{p}",
                        tag=f"w{kind}")
    nc.gpsimd.dma_start(
        w_big[:], w_dram[p].rearrange("(c q) j -> q c j", q=128))
    return w_big


def _project(nc, p, xt, aug_pool, w_big, mask_row, ps_m, kind):
    """Q^T/K^T pair projection -> two augmented per-head tiles [65, L]
    (rows 0-63: head data, row 64: ones / -BIGK*padk)."""
    wts = [w_big[:, c * 128:(c + 1) * 128] for c in range(NT)]
    pr_ps = ps_m.tile([128, L], F32, name=f"prps{kind}{p}", tag="m")
    for lh in range(2):
        for c in range(NT):
            nc.tensor.matmul(
                pr_ps[:, lh * 512:(lh + 1) * 512],
                _r(wts[c][:]),
                _r(xt[c][:, lh * 512:(lh + 1) * 512]),
                start=(c == 0), stop=(c == NT - 1))
    augs = []
    for hi in range(2):
        aug = aug_pool.tile([65, L], F32R, name=f"aug{kind}{p}_{hi}",
                            tag=f"aug{hi}")
        nc.vector.tensor_copy(aug[0:64, :],
                              pr_ps[hi * 64:(hi + 1) * 64, :])
        nc.gpsimd.dma_start(aug[64:65, :], mask_row)
        augs.append(aug)
    return augs


def _attend_pair(nc, p, qk, vsb,
                 p_pool, pt_pool, rr_pool, rsum_pool, rflat_pool,
                 ps_m, ps_o, attn_d, qbias_t, rmask_t, ones_col_t, catt_p,
                 fillers=(), preloads=()):
    fillers = list(fillers) if not isinstance(fillers, list) else fillers
    qaug, kaug = qk
    h0, h1 = 2 * p, 2 * p + 1

    rsum = [rsum_pool.tile([128, NT], F32, name=f"rsum{p}_{i}", tag=f"rsum{i}")
            for i in range(2)]
    rstar = [rsum_pool.tile([128, NT], F32, name=f"rstar{p}_{i}",
                            tag=f"rstar{i}")
             for i in range(2)]
    rstar_r = [rsum_pool.tile([128, NT], F32R, name=f"rstarr{p}_{i}",
                              tag=f"rstarr{i}")
               for i in range(2)]

    # ---- S -> exp -> A -> store (B phase) ----
    for hi in range(2):
        h = h0 if hi == 0 else h1
        for qt in range(NT):
            s_ps = ps_m.tile([128, L], F32, tag="m")
            for sh in range(2):
                nc.tensor.matmul(
                    s_ps[:, sh * 512:(sh + 1) * 512],
                    _r(qaug[hi][:, qt * 128:(qt + 1) * 128]),
                    _r(kaug[hi][:, sh * 512:(sh + 1) * 512]),
                    start=True, stop=True)
            p_t = p_pool.tile([128, L], F32)
            nc.scalar.activation(p_t[:], s_ps[:], EXP,
                                 bias=qbias_t[:, qt:qt + 1], scale=SCALE,
                                 accum_out=rsum[hi][:, qt:qt + 1])
            nc.vector.tensor_scalar_add(rstar[hi][:, qt:qt + 1],
                                        rsum[hi][:, qt:qt + 1], 1e-12)
            nc.vector.reciprocal(rstar[hi][:, qt:qt + 1],
                                 rstar[hi][:, qt:qt + 1])
            nc.vector.tensor_mul(rstar[hi][:, qt:qt + 1],
                                 rstar[hi][:, qt:qt + 1],
                                 rmask_t[:, qt:qt + 1])
            nc.vector.tensor_scalar_mul(p_t[:], p_t[:],
                                        rstar[hi][:, qt:qt + 1])
            nc.sync.dma_start(attn_d[h, qt * 128:(qt + 1) * 128, :],
                              p_t[:])
            if fillers and (hi, qt) >= (0, 2):
                fillers.pop(0)()

    # C phase start: issue prefetch DMAs for the next pair/quad plus the
    # r-flatten DMAs of this pair (the B phase saturates HBM with stores).
    rflat = [rflat_pool.tile([1, L], F32R, name=f"rflat{p}_{hi2}",
                             tag=f"rflat{hi2}")
             for hi2 in range(2)]

    def d_loads():
        for hi2 in range(2):
            nc.vector.tensor_copy(rstar_r[hi2][:], rstar[hi2][:])
            for t in range(NT):
                nc.gpsimd.dma_start(rflat[hi2][:, t * 128:(t + 1) * 128],
                                    rstar_r[hi2][:, t:t + 1])
    for f in preloads:
        f()
    d_loads()

    # ---- S^T -> exp -> P^T ; O^T accumulation (C phase; bf16 O^T packs
    # both heads into one tile via column tile_position). The O^T matmuls
    # for step st are emitted AFTER the S^T matmuls of st+1: engines issue
    # in order, so an O^T waiting on exp2(st) must not sit ahead of ready
    # S^T work in the PE stream. ----
    ot_ps = ps_o.tile([128, L], F32, name=f"ot{p}", tag="ot")
    pt_hist = {}

    def st_mms(st):
        st_ps = [ps_m.tile([128, L], F32, name=f"stps{p}_{st}_{i}", tag="m")
                 for i in range(2)]
        for hi in range(2):
            for qh in range(2):
                nc.tensor.matmul(
                    st_ps[hi][:, qh * 512:(qh + 1) * 512],
                    _r(kaug[hi][:, st * 128:(st + 1) * 128]),
                    _r(qaug[hi][:, qh * 512:(qh + 1) * 512]),
                    start=True, stop=True)
        pt_t = [pt_pool.tile([128, L], BF16, name=f"pt{p}_{st}_{i}",
                             tag=f"pt{i}")
                for i in range(2)]
        for hi in range(2):
            nc.scalar.activation(pt_t[hi][:], st_ps[hi][:], EXP,
                                 bias=0.0, scale=SCALE)
        pt_hist[st] = pt_t

    def ot_mms(st):
        pt_t = pt_hist.pop(st)
        for qh in range(2):
            for hi in range(2):
                nc.tensor.matmul(
                    ot_ps[hi * 64:(hi + 1) * 64, qh * 512:(qh + 1) * 512],
                    vsb[st][:, ((p % 2) * 2 + hi) * 64:
                            ((p % 2) * 2 + hi + 1) * 64],
                    pt_t[hi][:, qh * 512:(qh + 1) * 512],
                    start=(st == 0), stop=(st == NT - 1),
                    tile_position=(0, hi * 64))

    for st in range(NT):
        st_mms(st)
        if st >= 1:
            ot_mms(st - 1)

    # ---- normalize O^T by r (replicated along partitions). Deferred:
    # returned as closures the caller interleaves into the NEXT pair's B
    # phase, keeping this off the inter-pair critical path. ----
    def d_phase(hi):
        def run():
            rr_ps = ps_m.tile([64, L], F32, name=f"rrps{p}_{hi}", tag="m")
            for qh in range(2):
                nc.tensor.matmul(
                    rr_ps[:, qh * 512:(qh + 1) * 512],
                    _r(ones_col_t[:, :]),
                    _r(rflat[hi][:, qh * 512:(qh + 1) * 512]),
                    start=True, stop=True)
            rr_sb = rr_pool.tile([64, L], F32, name=f"rrsb{p}_{hi}",
                                 tag=f"rrsb{hi}")
            nc.vector.tensor_copy(rr_sb[:], rr_ps[:])
            nc.vector.tensor_mul(catt_p[hi * 64:(hi + 1) * 64, :],
                                 ot_ps[hi * 64:(hi + 1) * 64, :], rr_sb[:])
        return run

    def ot_tail():
        ot_mms(NT - 1)
    return [ot_tail, d_phase(0), d_phase(1)]


_CACHED = {}
_LAST_EXEC_NS = None
_LAST_RES = None


def _get_nc():
    if "nc" not in _CACHED:
        _CACHED["nc"] = build_nc()
    return _CACHED["nc"]


def _prep_core_inputs(Xi, WQT, WKT, WVT, WOT, pmi,
                      ones_row, ones_col):
    pm = pmi.astype(np.float32)
    pm_tiled = pm.reshape(NT, 128).T.copy()      # [128, NT]
    return {
        "xt": np.ascontiguousarray(Xi.T),
        "wqt": WQT, "wkt": WKT, "wvt": WVT, "wot": WOT,
        "qbias": np.ascontiguousarray(BIGQ * pm_tiled),
        "rmask": np.ascontiguousarray(1.0 - pm_tiled),
        "ones_row": ones_row,
        "kmask_row": np.ascontiguousarray((BIGK * pm)[None, :]),
        "ones_col": ones_col,
    }


def kernel(X, WQ, WK, WV, WO, pad_mask):
    X = np.asarray(X, dtype=np.float32)
    WQ = np.asarray(WQ, dtype=np.float32)
    WK = np.asarray(WK, dtype=np.float32)
    WV = np.asarray(WV, dtype=np.float32)
    WO = np.asarray(WO, dtype=np.float32)
    pad_mask = np.asarray(pad_mask)

    WQT = np.stack([np.concatenate([WQ[2 * p].T, WQ[2 * p + 1].T], axis=1)
                    for p in range(NP)])          # [NP, D, 128]
    WKT = np.stack([np.concatenate([WK[2 * p].T, WK[2 * p + 1].T], axis=1)
                    for p in range(NP)])
    WVT = np.stack([np.concatenate([WV[4 * g + j].T for j in range(4)], axis=1)
                    for g in range(NQ)])          # [NQ, D, 256]
    WOT = np.ascontiguousarray(WO.T)
    ones_row = np.ones((1, L), np.float32)
    ones_col = np.ones((1, 64), np.float32)

    in_maps = [
        _prep_core_inputs(X[i], WQT, WKT, WVT, WOT, pad_mask[i],
                          ones_row, ones_col)
        for i in range(B)
    ]

    nc = _get_nc()
    trace = bool(os.environ.get("KERNEL_TRACE"))
    kw = {}
    if trace:
        kw["trace"] = True
        kw["tmpdir"] = os.environ.get("KERNEL_TRACE_DIR") or None
    res = run_bass_kernel_spmd(nc, in_maps, list(range(B)), **kw)
    global _LAST_EXEC_NS, _LAST_RES
    _LAST_RES = res
    _LAST_EXEC_NS = res.exec_time_ns

    out = np.stack([res.results[i]["out"] for i in range(B)])
    attn = np.stack([res.results[i]["attn"] for i in range(B)])
    return out, attn


if __name__ == "__main__":
    rng = np.random.default_rng(0)
    X = rng.standard_normal((B, L, D), dtype=np.float32)
    WQ = (rng.standard_normal((H, DK, D), dtype=np.float32) * 0.02)
    WK = (rng.standard_normal((H, DK, D), dtype=np.float32) * 0.02)
    WV = (rng.standard_normal((H, DV, D), dtype=np.float32) * 0.02)
    WO = (rng.standard_normal((D, H * DV), dtype=np.float32) * 0.02)
    pm = rng.integers(0, 2, size=(B, L)).astype(bool)
    out, attn = kernel(X=X, WQ=WQ, WK=WK, WV=WV, WO=WO, pad_mask=pm)
    print("out", out.shape, "attn", attn.shape)


def bench(reps=16, **inputs):
    """Time repeated on-device executions with resident inputs.

    Outputs are fed back as the next iteration's donated output buffers
    (every output element is overwritten), so the loop moves no host data.
    Returns seconds per iteration.
    """
    import time
    import jax
    import numpy as jnp_np
    from jax.sharding import Mesh, PartitionSpec, NamedSharding
    from jax.experimental.shard_map import shard_map
    from concourse import bass2jax
    from concourse.bass2jax import _bass_exec_p, partition_id_tensor, \
        install_neuronx_cc_hook
    import concourse.mybir as mybir

    install_neuronx_cc_hook()
    nc = _get_nc()

    X = np.asarray(inputs["X"], dtype=np.float32)
    WQ = np.asarray(inputs["WQ"], dtype=np.float32)
    WK = np.asarray(inputs["WK"], dtype=np.float32)
    WV = np.asarray(inputs["WV"], dtype=np.float32)
    WO = np.asarray(inputs["WO"], dtype=np.float32)
    pad_mask = np.asarray(inputs["pad_mask"])
    WQT = np.stack([np.concatenate([WQ[2 * p].T, WQ[2 * p + 1].T], axis=1)
                    for p in range(NP)])
    WKT = np.stack([np.concatenate([WK[2 * p].T, WK[2 * p + 1].T], axis=1)
                    for p in range(NP)])
    WVT = np.stack([np.concatenate([WV[4 * g + j].T for j in range(4)], axis=1)
                    for g in range(NQ)])
    WOT = np.ascontiguousarray(WO.T)
    ones_row = np.ones((1, L), np.float32)
    ones_col = np.ones((1, 64), np.float32)
    in_maps = [_prep_core_inputs(X[i], WQT, WKT, WVT, WOT, pad_mask[i],
                                 ones_row, ones_col) for i in range(B)]

    partition_name = (nc.partition_id_tensor.name
                      if nc.partition_id_tensor else None)
    in_names, out_names, out_avals, zero_outs = [], [], [], []
    for alloc in nc.m.functions[0].allocations:
        if not isinstance(alloc, mybir.MemoryLocationSet):
            continue
        name = alloc.memorylocations[0].name
        if alloc.kind == "ExternalInput":
            if name != partition_name:
                in_names.append(name)
        elif alloc.kind == "ExternalOutput":
            out_names.append(name)
            shape = tuple(alloc.tensor_shape)
            dtype = mybir.dt.np(alloc.dtype)
            out_avals.append(jax.core.ShapedArray(shape, dtype))
            zero_outs.append(np.zeros(shape, dtype))
    n_params = len(in_names)
    n_outs = len(out_avals)
    all_in_names = list(in_names) + list(out_names)
    if partition_name is not None:
        all_in_names.append(partition_name)
    donate = tuple(range(n_params, n_params + n_outs))

    def _body(*args):
        operands = list(args)
        if partition_name is not None:
            operands.append(partition_id_tensor())
        outs = _bass_exec_p.bind(
            *operands,
            out_avals=tuple(out_avals),
            in_names=tuple(all_in_names),
            out_names=tuple(out_names),
            lowering_input_output_aliases=(),
            sim_require_finite=True,
            sim_require_nnan=True,
            nc=nc,
        )
        return tuple(outs)

    devices = jax.devices()[:B]
    mesh = Mesh(np.asarray(devices), ("core",))
    in_specs = (PartitionSpec("core"),) * (n_params + n_outs)
    out_specs = (PartitionSpec("core"),) * n_outs
    sharded = jax.jit(
        shard_map(_body, mesh=mesh, in_specs=in_specs,
                  out_specs=out_specs, check_rep=False),
        donate_argnums=donate, keep_unused=True)

    shard = NamedSharding(mesh, PartitionSpec("core"))
    concat_in = [
        jax.device_put(
            np.concatenate([np.asarray(in_maps[c][n]) for c in range(B)],
                           axis=0), shard)
        for n in in_names
    ]
    cur_outs = [
        jax.device_put(
            np.zeros((B * z.shape[0], *z.shape[1:]), z.dtype), shard)
        for z in zero_outs
    ]

    # warmup (also compiles)
    cur_outs = list(sharded(*concat_in, *cur_outs))
    jax.block_until_ready(cur_outs)

    t0 = time.time()
    for _ in range(reps):
        cur_outs = list(sharded(*concat_in, *cur_outs))
    jax.block_until_ready(cur_outs)
    t1 = time.time()
    return (t1 - t0) / reps
